# revision 15
# baseline (speedup 1.0000x reference)
"""Trainium2 Bass kernel for a fused MultiHead-GAT layer (8-core SPMD).

Strategy (edges sharded by sorted dst; tgt nodes data-parallel):
  host:  sort edges by dst, shard dst ranges across 8 cores; per 128-tgt
         block split edges into two halves by src row (AllGather chunk 1 /
         chunk 2), pad each half to uniform chunk counts, pre-transpose
         edge_embed / one-hot chunks to partition-major contiguous layout,
         fold attn_a into tiny weight matrices.
  device (per core):
    dummy 64B AllGather first (absorbs inter-core launch skew while z
    computes); z rows = src_h_shard @ W_fc (columns permuted o*8+h) +
    s1 = src_h @ (W_fc @ Ablk) -> bounce; TWO AllGathers (bounce rows
    0:640 -> zc1, 640:1250 -> zc2) so half-1 gathers start ~30us earlier.
    per tgt block: dma_gather z rows per edge (<=640 idxs per call),
    calls striped across 4 SWDGE queues (per-queue Q7 core pairs run
    desc-gen in parallel); s2 via PE (edge_embed^T @ V); e = leaky(s1+s2)
    per half; softmax without max-shift; aggregation via one-hot matmul
    accumulating [t, 512] + denom in PSUM; divide+unpermute; elu via
    scalar engine (exp(min(x,0)) = exp(-relu(-x))) + gpsimd fused
    max/add; FFN: a1 hid-major, o2 flipped to NODE-major (stationary =
    r1 chunk) so +bias+residual+LayerNorm run without transposes;
    LayerNorm via bn_stats/bn_aggr; f32 output.
    FFN chunks interleave into the block loop to fill gather gaps.
"""
import sys

sys.path.insert(0, "/opt/trn_rl_repo")

from contextlib import ExitStack
from types import SimpleNamespace

import numpy as np
import ml_dtypes

import concourse.bass as bass
import concourse.bacc as bacc
import concourse.tile as tile
from concourse import mybir

BF16 = mybir.dt.bfloat16
F32 = mybir.dt.float32
I16 = mybir.dt.int16
NP_BF16 = ml_dtypes.bfloat16

LN_EPS = 1e-5
LEAK = 0.01

GMAX = 5        # chunks (x128 idxs) per dma_gather call; 640 idxs proven on HW
NQUEUES = 4     # stripe gather calls across SWDGE queues (Q7 core pairs)
RSPLIT = 1024   # z-table AllGather split: bounce rows [0:RSPLIT] / [RSPLIT:]


def full_cfg():
    return SimpleNamespace(
        ncores=8,
        n_src=10000, n_tgt=10000, e=160000,
        in_dim=512, d=512, h=8, o=64, ed=128, fh=2048,
        tgt_per=1250, tgt_pad=1280, nblk=10,
        zrow=640,  # 512 z + 8 s1 + 120 pad (row bytes % 256 == 0)
    )


def host_prep(cfg, src_h, tgt_h, edge_embed, edge_src, edge_dst,
              W_fc, W_feat, attn_a, w1, b1, w2, b2, ln_g, ln_b):
    C = cfg
    H, O, D = C.h, C.o, C.d

    perm = np.argsort(edge_dst, kind="stable")
    es = np.asarray(edge_src)[perm].astype(np.int64)
    ed = np.asarray(edge_dst)[perm].astype(np.int64)
    ee = np.asarray(edge_embed)[perm]

    # feature permutation q = o*8+h  <->  f = h*64+o
    q = np.arange(D)
    f_of_q = (q % H) * O + (q // H)          # column f placed at position q
    Wfc_p = np.asarray(W_fc)[:, f_of_q]       # z_perm = src_h @ Wfc_p

    a_src = np.asarray(attn_a)[0, :, :O]       # [H, O]
    a_feat = np.asarray(attn_a)[0, :, 2 * O:]  # [H, O]
    Ablk = np.zeros((D, H), np.float32)
    for h in range(H):
        Ablk[h * O:(h + 1) * O, h] = a_src[h]
    M1 = (np.asarray(W_fc, np.float64) @ Ablk.astype(np.float64)).astype(np.float32)  # [D, H]
    V = np.zeros((C.ed, H), np.float32)
    for h in range(H):
        V[:, h] = np.asarray(W_feat)[:, h * O:(h + 1) * O] @ a_feat[h]

    # global src id -> (table, local row): core c = s // tgt_per, r = s % tgt_per
    # table 1 rows: RSPLIT per core; table 2: tgt_per - RSPLIT per core.
    R2 = C.tgt_per - RSPLIT

    def table_row(s):
        c, r = s // C.tgt_per, s % C.tgt_per
        if r < RSPLIT:
            return 0, RSPLIT * c + r
        return 1, R2 * c + (r - RSPLIT)

    # per-core edge partition by per-core 128-tgt blocks, split by src half
    block_bounds = []
    for c in range(C.ncores):
        for b in range(C.nblk):
            lo = c * C.tgt_per + b * 128
            hi = min(lo + 128, (c + 1) * C.tgt_per)
            block_bounds.append((lo, hi))

    halves = []   # per (core*nblk + b): (src1, lt1, ee1, src2, lt2, ee2)
    cnt1m = cnt2m = 1
    for (lo, hi) in block_bounds:
        s, t = np.searchsorted(ed, lo), np.searchsorted(ed, hi)
        srcs, dsts, ees = es[s:t], ed[s:t] - lo, ee[s:t]
        r = srcs % C.tgt_per
        m1_ = r < RSPLIT
        halves.append((srcs[m1_], dsts[m1_], ees[m1_],
                       srcs[~m1_], dsts[~m1_], ees[~m1_]))
        cnt1m = max(cnt1m, int(m1_.sum()))
        cnt2m = max(cnt2m, int((~m1_).sum()))
    cpb1 = (cnt1m + 127) // 128
    cpb2 = (cnt2m + 127) // 128
    C.cpb1, C.cpb2 = cpb1, cpb2
    C.cpb = cpb1 + cpb2
    cpb = C.cpb

    def pack_half(srcs, dsts, ees, nch, tbl):
        n = len(srcs)
        src_b = np.zeros(nch * 128, np.int64)
        if tbl == 0:
            loc = (srcs // C.tgt_per) * RSPLIT + (srcs % C.tgt_per)
        else:
            loc = (srcs // C.tgt_per) * R2 + (srcs % C.tgt_per - RSPLIT)
        src_b[:n] = loc
        lt = np.full(nch * 128, -1, np.int64)
        lt[:n] = dsts
        eb = np.zeros((nch * 128, C.ed), NP_BF16)
        eb[:n] = ees.astype(NP_BF16)
        ohb = np.zeros((nch * 128, 128), NP_BF16)
        valid = lt >= 0
        ohb[np.nonzero(valid)[0], lt[valid]] = 1.0
        return (src_b.astype(np.int16).reshape(-1, 16).T,     # [16, nch*8]
                eb.reshape(nch, 128, C.ed).transpose(2, 0, 1),  # [ed, nch, 128]
                ohb.reshape(nch, 128, 128).transpose(1, 0, 2))  # [e, nch, t]

    cores = []
    for c in range(C.ncores):
        idxw = np.zeros((128, C.nblk, cpb * 8), np.int16)
        eeT = np.zeros((C.nblk, 128, cpb, 128), NP_BF16)
        oh = np.zeros((C.nblk, 128, cpb, 128), NP_BF16)
        for b in range(C.nblk):
            s1a, d1a, e1a, s2a, d2a, e2a = halves[c * C.nblk + b]
            i1, ee1, oh1 = pack_half(s1a, d1a, e1a, cpb1, 0)
            i2, ee2, oh2 = pack_half(s2a, d2a, e2a, cpb2, 1)
            base = np.concatenate([i1, i2], axis=1)   # [16, cpb*8]
            for k in range(8):
                idxw[k * 16:(k + 1) * 16, b, :] = base
            eeT[b, :, :cpb1], eeT[b, :, cpb1:] = ee1, ee2
            oh[b, :, :cpb1], oh[b, :, cpb1:] = oh1, oh2

        th = np.zeros((C.tgt_pad, D), np.float32)
        lo = c * C.tgt_per
        hi = min((c + 1) * C.tgt_per, C.n_tgt)
        th[:hi - lo] = np.asarray(tgt_h)[lo:hi] - 1.0   # elu trick: + (tgt_h - 1)
        sh = np.zeros((C.in_dim, C.tgt_pad), np.float32)
        lo2 = c * C.tgt_per
        hi2 = min((c + 1) * C.tgt_per, C.n_src)
        sh[:, :hi2 - lo2] = np.asarray(src_h)[lo2:hi2].T

        cores.append({
            "idxw": idxw, "eeT": eeT, "oh": oh,
            "tgt_hm1": th.astype(NP_BF16),
            "src_hT": sh.astype(NP_BF16),
        })

    shared = {
        "wfc": Wfc_p.astype(NP_BF16),
        "m1": M1.astype(NP_BF16),
        "v": V.astype(NP_BF16),
        "w1": np.asarray(w1).astype(NP_BF16),
        "w2": np.asarray(w2).astype(NP_BF16),
        "b1c": np.asarray(b1, np.float32).reshape(C.fh, 1),
        "b2_rep": np.tile(np.asarray(b2, np.float32).reshape(1, D), (128, 1)),
        "g_rep": np.tile(np.asarray(ln_g, np.float32).reshape(1, D), (128, 1)),
        "b_rep": np.tile(np.asarray(ln_b, np.float32).reshape(1, D), (128, 1)),
        "identb": np.eye(128, dtype=NP_BF16),
    }
    return cores, shared


def build_program(C):
    nc = bacc.Bacc("TRN2", target_bir_lowering=False, debug=False,
                   num_devices=C.ncores, num_swdge_queues=NQUEUES)
    H, O, D, NBLK = C.h, C.o, C.d, C.nblk
    CPB1, CPB2, CPB = C.cpb1, C.cpb2, C.cpb
    ZR = C.zrow
    NPAD = C.tgt_pad
    R2 = C.tgt_per - RSPLIT

    # -------- I/O --------
    def din(name, shape, dt):
        return nc.dram_tensor(name, shape, dt, kind="ExternalInput").ap()

    idxw = din("idxw", [128, NBLK, CPB * 8], I16)
    eeT = din("eeT", [NBLK, 128, CPB, 128], BF16)
    oh = din("oh", [NBLK, 128, CPB, 128], BF16)
    tgt_hm1 = din("tgt_hm1", [NPAD, D], BF16)
    src_hT = din("src_hT", [C.in_dim, NPAD], BF16)
    wfc = din("wfc", [C.in_dim, D], BF16)
    m1 = din("m1", [C.in_dim, H], BF16)
    vmat = din("v", [C.ed, H], BF16)
    w1 = din("w1", [D, C.fh], BF16)
    w2 = din("w2", [C.fh, D], BF16)
    b1c = din("b1c", [C.fh, 1], F32)
    b2_rep = din("b2_rep", [128, D], F32)
    g_rep = din("g_rep", [128, D], F32)
    b_rep = din("b_rep", [128, D], F32)
    identb = din("identb", [128, 128], BF16)

    out_shard = nc.dram_tensor("out_shard", [NPAD, D], F32,
                               kind="ExternalOutput").ap()

    zc_bounce = nc.dram_tensor("zc_bounce", [C.tgt_per, ZR], BF16).ap()
    zc_space = "Shared" if C.ncores > 4 else None
    zc1 = nc.dram_tensor("zc1", [C.ncores * RSPLIT, ZR], BF16,
                         addr_space=zc_space).ap()
    zc2 = nc.dram_tensor("zc2", [C.ncores * R2, ZR], BF16,
                         addr_space=zc_space).ap()
    dummy_out = nc.dram_tensor("dummy_out", [C.ncores, 32], BF16,
                               addr_space=zc_space).ap()
    dummy_src = nc.dram_tensor("dummy_src", [1, 32], BF16).ap()

    KT = C.in_dim // 128   # 4
    FT = D // 128          # 4
    MT1 = C.fh // 128      # 16
    chunks = [(0, 384), (384, 512), (896, 256), (1152, 128)]

    with tile.TileContext(nc) as tc, ExitStack() as top:
        const = top.enter_context(tc.tile_pool(name="const", bufs=1))

        # skew-absorbing dummy collective
        dsrc_sb = const.tile([1, 32], BF16)
        nc.vector.memset(dsrc_sb[:], 0.0)
        nc.sync.dma_start(dummy_src[:, :], dsrc_sb[:])
        nc.gpsimd.collective_compute(
            "AllGather", mybir.AluOpType.bypass,
            replica_groups=[list(range(C.ncores))],
            ins=[dummy_src[:, :]], outs=[dummy_out[:, :]],
        )

        wfc_sb = const.tile([128, KT, D], BF16)
        nc.sync.dma_start(wfc_sb[:], wfc.rearrange("(kt p) m -> p kt m", p=128))
        m1_sb = const.tile([128, KT, H], BF16)
        nc.sync.dma_start(m1_sb[:], m1.rearrange("(kt p) m -> p kt m", p=128))
        v_sb = const.tile([128, H], BF16)
        nc.sync.dma_start(v_sb[:], vmat[:, :])
        idx_sb = const.tile([128, NBLK, CPB * 8], I16)
        nc.sync.dma_start(idx_sb[:], idxw[:, :, :])
        w1_sb = const.tile([128, KT, MT1, 128], BF16)
        w2_sb = const.tile([128, MT1, FT, 128], BF16)
        b1_sb = const.tile([128, MT1, 1], F32)
        b2r_sb = const.tile([128, D], F32)
        grep_sb = const.tile([128, D], F32)
        brep_sb = const.tile([128, D], F32)
        idb_sb = const.tile([128, 128], BF16)
        zero_sb = const.tile([128, 1], F32)
        nc.vector.memset(zero_sb[:], 0.0)
        eps_sb = const.tile([128, 1], F32)
        nc.vector.memset(eps_sb[:], LN_EPS)

        # ---------------- phase 0: z rows + s1 rows -> zc_bounce -> AllGather
        with ExitStack() as p0:
            ps0 = p0.enter_context(tc.tile_pool(name="ps0", bufs=2, space="PSUM"))
            zr_pool = p0.enter_context(tc.tile_pool(name="zrow", bufs=3))
            shp = p0.enter_context(tc.tile_pool(name="shp", bufs=1))
            sh_sb = shp.tile([128, KT, NPAD], BF16)
            nc.sync.dma_start(sh_sb[:],
                              src_hT.rearrange("(kt p) n -> p kt n", p=128))
            for nb in range(NBLK):
                rows = min(128, C.tgt_per - nb * 128)
                if rows <= 0:
                    break
                z_ps = ps0.tile([128, D], F32, tag="zps")
                for kt in range(KT):
                    nc.tensor.matmul(z_ps[:], sh_sb[:, kt, nb * 128:(nb + 1) * 128],
                                     wfc_sb[:, kt, :], start=(kt == 0),
                                     stop=(kt == KT - 1))
                s1_ps = ps0.tile([128, H], F32, tag="s1ps")
                for kt in range(KT):
                    nc.tensor.matmul(s1_ps[:], sh_sb[:, kt, nb * 128:(nb + 1) * 128],
                                     m1_sb[:, kt, :], start=(kt == 0),
                                     stop=(kt == KT - 1))
                zrow = zr_pool.tile([128, D + H], BF16, tag="zrow")
                nc.vector.tensor_copy(zrow[:, 0:D], z_ps[:])
                nc.vector.tensor_copy(zrow[:, D:D + H], s1_ps[:])
                nc.sync.dma_start(zc_bounce[nb * 128:nb * 128 + rows, 0:D + H],
                                  zrow[0:rows, :])

        nc.gpsimd.collective_compute(
            "AllGather", mybir.AluOpType.bypass,
            replica_groups=[list(range(C.ncores))],
            ins=[zc_bounce[0:RSPLIT, :]], outs=[zc1[:, :]],
        )
        nc.gpsimd.collective_compute(
            "AllGather", mybir.AluOpType.bypass,
            replica_groups=[list(range(C.ncores))],
            ins=[zc_bounce[RSPLIT:C.tgt_per, :]], outs=[zc2[:, :]],
        )

        # FFN/LN consts load after phase 0 so they don't delay the z pipeline
        nc.sync.dma_start(w1_sb[:], w1.rearrange("(kt p) (mt m) -> p kt mt m",
                                                 p=128, m=128))
        nc.sync.dma_start(w2_sb[:], w2.rearrange("(kt p) (mt m) -> p kt mt m",
                                                 p=128, m=128))
        nc.sync.dma_start(b1_sb[:], b1c.rearrange("(mt p) x -> p mt x", p=128))
        nc.sync.dma_start(b2r_sb[:], b2_rep[:, :])
        nc.sync.dma_start(grep_sb[:], g_rep[:, :])
        nc.sync.dma_start(brep_sb[:], b_rep[:, :])
        nc.sync.dma_start(idb_sb[:], identb[:, :])

        # ---------------- blocks + FFN, interleaved so PE fills gather gaps
        with ExitStack() as pb:
            ps_s2 = pb.enter_context(tc.tile_pool(name="ps_s2", bufs=1, space="PSUM"))
            ps_hag = pb.enter_context(tc.tile_pool(name="ps_hag", bufs=2, space="PSUM"))
            ps_tp = pb.enter_context(tc.tile_pool(name="ps_tp", bufs=1, space="PSUM"))
            ps_a1 = pb.enter_context(tc.tile_pool(name="ps_a1", bufs=1, space="PSUM"))
            ps_o2 = pb.enter_context(tc.tile_pool(name="ps_o2", bufs=1, space="PSUM"))
            gpool = pb.enter_context(tc.tile_pool(name="zg", bufs=2))
            zgsp = pb.enter_context(tc.tile_pool(name="zgs", bufs=2))
            epool = pb.enter_context(tc.tile_pool(name="escore", bufs=2))
            hpool = pb.enter_context(tc.tile_pool(name="hb", bufs=1))
            hkeep = pb.enter_context(tc.tile_pool(name="hb3", bufs=5))
            tgtp = pb.enter_context(tc.tile_pool(name="tgtp", bufs=2))
            hbtp = pb.enter_context(tc.tile_pool(name="hbt", bufs=1))
            r1p = pb.enter_context(tc.tile_pool(name="r1", bufs=1))
            tmpp = pb.enter_context(tc.tile_pool(name="tmp", bufs=2))
            lnp = pb.enter_context(tc.tile_pool(name="ln", bufs=1))
            stp = pb.enter_context(tc.tile_pool(name="stat", bufs=2))

            hbT = hbtp.tile([128, FT, NPAD], BF16)
            hb3_tiles = {}
            zg_tiles = {}

            qsel = [0]

            def emit_gathers(nb, half):
                if nb not in zg_tiles:
                    zg = gpool.tile([128, CPB, ZR], BF16, tag="zg")
                    zg_tiles[nb] = zg
                zg = zg_tiles[nb]
                lo, hi = (0, CPB1) if half == 0 else (CPB1, CPB)
                tab = zc1 if half == 0 else zc2
                for g0 in range(lo, hi, GMAX):
                    gn = min(GMAX, hi - g0)
                    nc.gpsimd.dma_gather(
                        out_ap=zg[:, g0:g0 + gn, :], in_ap=tab[:, :],
                        idxs_ap=idx_sb[:, nb, g0 * 8:(g0 + gn) * 8],
                        num_idxs=gn * 128, num_idxs_reg=gn * 128,
                        elem_size=ZR, queue_num=qsel[0] % NQUEUES)
                    qsel[0] += 1

            def emit_ffn_chunk(cs, cw):
                r1 = r1p.tile([128, MT1, cw], BF16, tag="r1")
                for mt in range(MT1):
                    a1 = ps_a1.tile([128, cw], F32, tag="a1")
                    for kt in range(KT):
                        nc.tensor.matmul(a1[:], w1_sb[:, kt, mt, :],
                                         hbT[:, kt, cs:cs + cw],
                                         start=(kt == 0), stop=(kt == KT - 1))
                    if mt % 2 == 0:
                        nc.scalar.activation(r1[:, mt, :], a1[:],
                                             mybir.ActivationFunctionType.Relu,
                                             bias=b1_sb[:, mt, :])
                    else:
                        nc.vector.tensor_scalar(r1[:, mt, :], a1[:],
                                                b1_sb[:, mt, :], 0.0,
                                                mybir.AluOpType.add,
                                                mybir.AluOpType.max)
                # o2 node-major: stationary = r1 node-chunk, moving = w2
                for ns in range(cw // 128):
                    nb_ln = (cs + ns * 128) // 128
                    o2 = ps_o2.tile([128, D], F32, tag="o2")
                    for kt2 in range(MT1):
                        nc.tensor.matmul(
                            o2[:], r1[:, kt2, ns * 128:(ns + 1) * 128],
                            w2_sb[:, kt2, :, :],
                            start=(kt2 == 0), stop=(kt2 == MT1 - 1))
                    # r2n = (o2 + b2) + hb3   (ffn out + bias + residual)
                    t1 = tmpp.tile([128, D], F32, tag="t1")
                    nc.vector.tensor_tensor(t1[:], o2[:], b2r_sb[:],
                                            mybir.AluOpType.add)
                    r2n = lnp.tile([128, D], F32, tag="r2n")
                    nc.vector.tensor_tensor(r2n[:], t1[:],
                                            hb3_tiles.pop(nb_ln)[:],
                                            mybir.AluOpType.add)
                    # LayerNorm via bn_stats
                    st6 = stp.tile([128, 6], F32, tag="st6")
                    nc.vector.bn_stats(st6[:], r2n[:])
                    mv = stp.tile([128, 2], F32, tag="mv")
                    nc.vector.bn_aggr(mv[:], st6[:])
                    std = stp.tile([128, 1], F32, tag="std")
                    nc.scalar.activation(std[:], mv[:, 1:2],
                                         mybir.ActivationFunctionType.Sqrt,
                                         bias=eps_sb[:, :])
                    rstd = stp.tile([128, 1], F32, tag="rstd")
                    nc.vector.reciprocal(rstd[:], std[:])
                    xn = lnp.tile([128, D], BF16, tag="xn")
                    nc.vector.tensor_scalar(xn[:], r2n[:], mv[:, 0:1], rstd[:],
                                            mybir.AluOpType.subtract,
                                            mybir.AluOpType.mult)
                    xg = lnp.tile([128, D], F32, tag="xg")
                    nc.vector.scalar_tensor_tensor(
                        xg[:], xn[:], 1.0, grep_sb[:],
                        mybir.AluOpType.mult, mybir.AluOpType.mult)
                    orow = lnp.tile([128, D], F32, tag="orow")
                    nc.vector.tensor_tensor(orow[:], xg[:], brep_sb[:],
                                            mybir.AluOpType.add)
                    nc.sync.dma_start(out_shard[nb_ln * 128:(nb_ln + 1) * 128, :],
                                      orow[:])

            def emit_block(nb):
                zg = zg_tiles[nb]
                ee_t = gpool.tile([128, CPB, 128], BF16, tag="ee")
                nc.sync.dma_start(ee_t[:], eeT[nb])
                oh_t = gpool.tile([128, CPB, 128], BF16, tag="oh")
                nc.sync.dma_start(oh_t[:], oh[nb])
                tgtb = tgtp.tile([128, D], BF16, tag="tgtb")
                nc.sync.dma_start(tgtb[:], tgt_hm1[nb * 128:(nb + 1) * 128, :])

                s2_ps = ps_s2.tile([128, CPB * H], F32, tag="s2")
                for j in range(CPB):
                    nc.tensor.matmul(s2_ps[:, j * H:(j + 1) * H], ee_t[:, j, :],
                                     v_sb[:, :], start=True, stop=True)
                e1 = epool.tile([128, CPB, H], F32, tag="e1")
                e2 = epool.tile([128, CPB, H], F32, tag="e2")
                eexp = epool.tile([128, CPB, H], BF16, tag="eexp")
                zgs = zgsp.tile([128, CPB, D], BF16, tag="zgs")
                for (lo, hi) in ((0, CPB1), (CPB1, CPB)):
                    w = hi - lo
                    nc.vector.tensor_tensor(
                        e1[:, lo:hi, :],
                        s2_ps[:, lo * H:hi * H].rearrange("p (c h) -> p c h", h=H),
                        zg[:, lo:hi, D:D + H],
                        mybir.AluOpType.add)
                    # leaky relu: max(x, 0.01*x) fused
                    nc.vector.scalar_tensor_tensor(
                        e2[:, lo:hi, :], e1[:, lo:hi, :], LEAK, e1[:, lo:hi, :],
                        mybir.AluOpType.mult, mybir.AluOpType.max)
                    nc.scalar.activation(eexp[:, lo:hi, :], e2[:, lo:hi, :],
                                         mybir.ActivationFunctionType.Exp,
                                         bias=zero_sb[:, :])
                    nc.vector.tensor_tensor(
                        zgs[:, lo:hi, :].rearrange("p c (o h) -> p c o h", h=H),
                        zg[:, lo:hi, 0:D].rearrange("p c (o h) -> p c o h", h=H),
                        eexp[:, lo:hi, :].rearrange("p c (h x) -> p c x h", x=1)
                            .broadcast_to([128, w, O, H]),
                        mybir.AluOpType.mult)

                hag = ps_hag.tile([128, D + H], F32, tag="hag")
                for j in range(CPB):
                    nc.tensor.matmul(hag[:, 0:D], oh_t[:, j, :], zgs[:, j, :],
                                     start=(j == 0), stop=(j == CPB - 1),
                                     skip_group_check=True)
                    nc.tensor.matmul(hag[:, D:D + H], oh_t[:, j, :], eexp[:, j, :],
                                     start=(j == 0), stop=(j == CPB - 1),
                                     skip_group_check=True)

                den = epool.tile([128, H], F32, tag="den")
                nc.vector.tensor_scalar_max(den[:], hag[:, D:D + H], 1e-30)
                rec = epool.tile([128, H], F32, tag="rec")
                nc.vector.reciprocal(rec[:], den[:])

                hbp = hpool.tile([128, D], BF16, tag="hbp")
                nc.vector.tensor_tensor(
                    hbp[:, :].rearrange("p (h o) -> p h o", o=O),
                    hag[:, 0:D].rearrange("p (o h) -> p h o", h=H),
                    rec[:, :].rearrange("p (h x) -> p h x", x=1)
                        .broadcast_to([128, H, O]),
                    mybir.AluOpType.mult)
                # elu(x) + tgt = max(x,0) + exp(min(x,0)) + (tgt-1)
                # exp(min(x,0)) = exp(-relu(-x)), both on the scalar engine
                nrx = hpool.tile([128, D], BF16, tag="nrx")
                nc.scalar.activation(nrx[:], hbp[:],
                                     mybir.ActivationFunctionType.Relu,
                                     bias=zero_sb[:, :], scale=-1.0)
                ex = hpool.tile([128, D], BF16, tag="ex")
                nc.scalar.activation(ex[:], nrx[:],
                                     mybir.ActivationFunctionType.Exp,
                                     bias=zero_sb[:, :], scale=-1.0)
                hb23 = hpool.tile([128, D], BF16, tag="hb23")
                nc.vector.scalar_tensor_tensor(
                    hb23[:], hbp[:], 0.0, tgtb[:],
                    mybir.AluOpType.max, mybir.AluOpType.add)
                hb3 = hkeep.tile([128, D], BF16, tag="hb3")
                hb3_tiles[nb] = hb3
                nc.vector.tensor_tensor(hb3[:], hb23[:], ex[:],
                                        mybir.AluOpType.add)
                for ft in range(FT):
                    tpb = ps_tp.tile([128, 128], BF16, tag="tp")
                    nc.tensor.transpose(tpb[:], hb3[:, ft * 128:(ft + 1) * 128],
                                        idb_sb[:])
                    nc.vector.tensor_copy(hbT[:, ft, nb * 128:(nb + 1) * 128],
                                          tpb[:])

            # pipeline: prefetch both blocks' half-1 gathers before block 0's
            # half-2 (which waits on the second AllGather)
            emit_gathers(0, 0)
            emit_gathers(1, 0)
            emit_gathers(0, 1)
            next_chunk = 0
            for nb in range(NBLK):
                if nb == 1:
                    emit_gathers(1, 1)
                elif nb >= 2:
                    emit_gathers(nb, 0)
                    emit_gathers(nb, 1)
                emit_block(nb)
                del zg_tiles[nb]

                while (next_chunk < len(chunks)
                       and chunks[next_chunk][0] + chunks[next_chunk][1]
                       <= (nb + 1) * 128):
                    cs, cw = chunks[next_chunk]
                    emit_ffn_chunk(cs, cw)
                    next_chunk += 1

    nc.compile()
    return nc


_CACHE = {}


def _get_program(C):
    key = (C.ncores, C.n_src, C.n_tgt, C.e, C.cpb1, C.cpb2)
    if key not in _CACHE:
        _CACHE[key] = build_program(C)
    return _CACHE[key]


def kernel(src_h, tgt_h, edge_embed, edge_src, edge_dst,
           W_fc, W_feat, attn_a, w1, b1, w2, b2, ln_g, ln_b):
    from concourse.bass_utils import run_bass_kernel_spmd

    C = full_cfg()
    cores, shared = host_prep(C, src_h, tgt_h, edge_embed, edge_src, edge_dst,
                              W_fc, W_feat, attn_a, w1, b1, w2, b2, ln_g, ln_b)
    nc = _get_program(C)
    in_maps = []
    for c in range(C.ncores):
        m = dict(shared)
        cc = cores[c]
        m.update(idxw=cc["idxw"], eeT=cc["eeT"], oh=cc["oh"],
                 tgt_hm1=cc["tgt_hm1"], src_hT=cc["src_hT"])
        in_maps.append(m)
    import os
    try:
        res = run_bass_kernel_spmd(nc, in_maps, list(range(C.ncores)))
    except Exception:
        if os.environ.get("BASS_TRACE"):
            os.environ["BASS_NEVER_TRACE"] = "1"
            res = run_bass_kernel_spmd(nc, in_maps, list(range(C.ncores)))
        else:
            raise
    global _last_results
    _last_results = res
    out = np.concatenate(
        [res.results[c]["out_shard"][:C.tgt_per] for c in range(C.ncores)], axis=0)
    return np.ascontiguousarray(out, dtype=np.float32)


# revision 16
# speedup vs baseline: 1.0251x; 1.0251x over previous
"""Trainium2 Bass kernel for a fused MultiHead-GAT layer (8-core SPMD).

Strategy (edges sharded by sorted dst; tgt nodes data-parallel):
  host:  sort edges by dst, shard dst ranges across 8 cores; per 128-tgt
         block split edges into two halves by src row (AllGather chunk 1 /
         chunk 2), pad each half to uniform chunk counts, pre-transpose
         edge_embed / one-hot chunks to partition-major contiguous layout,
         fold attn_a into tiny weight matrices.
  device (per core):
    dummy 64B AllGather first (absorbs inter-core launch skew while z
    computes); z rows = src_h_shard @ W_fc (columns permuted o*8+h) +
    s1 = src_h @ (W_fc @ Ablk) -> bounce; TWO AllGathers (bounce rows
    0:640 -> zc1, 640:1250 -> zc2) so half-1 gathers start ~30us earlier.
    per tgt block: dma_gather z rows per edge (<=640 idxs per call),
    calls striped across 4 SWDGE queues (per-queue Q7 core pairs run
    desc-gen in parallel); s2 via PE (edge_embed^T @ V); e = leaky(s1+s2)
    per half; softmax without max-shift; aggregation via one-hot matmul
    accumulating [t, 512] + denom in PSUM; divide+unpermute; elu via
    scalar engine (exp(min(x,0)) = exp(-relu(-x))) + gpsimd fused
    max/add; FFN: a1 hid-major, o2 flipped to NODE-major (stationary =
    r1 chunk) so +bias+residual+LayerNorm run without transposes;
    LayerNorm via bn_stats/bn_aggr; f32 output.
    FFN chunks interleave into the block loop to fill gather gaps.
"""
import sys

sys.path.insert(0, "/opt/trn_rl_repo")

from contextlib import ExitStack
from types import SimpleNamespace

import numpy as np
import ml_dtypes

import concourse.bass as bass
import concourse.bacc as bacc
import concourse.tile as tile
from concourse import mybir

BF16 = mybir.dt.bfloat16
F32 = mybir.dt.float32
I16 = mybir.dt.int16
NP_BF16 = ml_dtypes.bfloat16

LN_EPS = 1e-5
LEAK = 0.01

GMAX = 5        # chunks (x128 idxs) per dma_gather call; 640 idxs proven on HW
NQUEUES = 4     # stripe gather calls across SWDGE queues (Q7 core pairs)
RSPLIT = 640    # z-table AllGather split: bounce rows [0:RSPLIT] / [RSPLIT:]


def full_cfg():
    return SimpleNamespace(
        ncores=8,
        n_src=10000, n_tgt=10000, e=160000,
        in_dim=512, d=512, h=8, o=64, ed=128, fh=2048,
        tgt_per=1250, tgt_pad=1280, nblk=10,
        zrow=640,  # 512 z + 8 s1 + 120 pad (row bytes % 256 == 0)
    )


def host_prep(cfg, src_h, tgt_h, edge_embed, edge_src, edge_dst,
              W_fc, W_feat, attn_a, w1, b1, w2, b2, ln_g, ln_b):
    C = cfg
    H, O, D = C.h, C.o, C.d

    perm = np.argsort(edge_dst, kind="stable")
    es = np.asarray(edge_src)[perm].astype(np.int64)
    ed = np.asarray(edge_dst)[perm].astype(np.int64)
    ee = np.asarray(edge_embed)[perm]

    # feature permutation q = o*8+h  <->  f = h*64+o
    q = np.arange(D)
    f_of_q = (q % H) * O + (q // H)          # column f placed at position q
    Wfc_p = np.asarray(W_fc)[:, f_of_q]       # z_perm = src_h @ Wfc_p

    a_src = np.asarray(attn_a)[0, :, :O]       # [H, O]
    a_feat = np.asarray(attn_a)[0, :, 2 * O:]  # [H, O]
    Ablk = np.zeros((D, H), np.float32)
    for h in range(H):
        Ablk[h * O:(h + 1) * O, h] = a_src[h]
    M1 = (np.asarray(W_fc, np.float64) @ Ablk.astype(np.float64)).astype(np.float32)  # [D, H]
    V = np.zeros((C.ed, H), np.float32)
    for h in range(H):
        V[:, h] = np.asarray(W_feat)[:, h * O:(h + 1) * O] @ a_feat[h]

    # global src id -> (table, local row): core c = s // tgt_per, r = s % tgt_per
    # table 1 rows: RSPLIT per core; table 2: tgt_per - RSPLIT per core.
    R2 = C.tgt_per - RSPLIT

    def table_row(s):
        c, r = s // C.tgt_per, s % C.tgt_per
        if r < RSPLIT:
            return 0, RSPLIT * c + r
        return 1, R2 * c + (r - RSPLIT)

    # per-core edge partition by per-core 128-tgt blocks, split by src half
    block_bounds = []
    for c in range(C.ncores):
        for b in range(C.nblk):
            lo = c * C.tgt_per + b * 128
            hi = min(lo + 128, (c + 1) * C.tgt_per)
            block_bounds.append((lo, hi))

    halves = []   # per (core*nblk + b): (src1, lt1, ee1, src2, lt2, ee2)
    cnt1m = cnt2m = 1
    for (lo, hi) in block_bounds:
        s, t = np.searchsorted(ed, lo), np.searchsorted(ed, hi)
        srcs, dsts, ees = es[s:t], ed[s:t] - lo, ee[s:t]
        r = srcs % C.tgt_per
        m1_ = r < RSPLIT
        halves.append((srcs[m1_], dsts[m1_], ees[m1_],
                       srcs[~m1_], dsts[~m1_], ees[~m1_]))
        cnt1m = max(cnt1m, int(m1_.sum()))
        cnt2m = max(cnt2m, int((~m1_).sum()))
    cpb1 = (cnt1m + 127) // 128
    cpb2 = (cnt2m + 127) // 128
    C.cpb1, C.cpb2 = cpb1, cpb2
    C.cpb = cpb1 + cpb2
    cpb = C.cpb

    def pack_half(srcs, dsts, ees, nch, tbl):
        n = len(srcs)
        src_b = np.zeros(nch * 128, np.int64)
        if tbl == 0:
            loc = (srcs // C.tgt_per) * RSPLIT + (srcs % C.tgt_per)
        else:
            loc = (srcs // C.tgt_per) * R2 + (srcs % C.tgt_per - RSPLIT)
        src_b[:n] = loc
        lt = np.full(nch * 128, -1, np.int64)
        lt[:n] = dsts
        eb = np.zeros((nch * 128, C.ed), NP_BF16)
        eb[:n] = ees.astype(NP_BF16)
        ohb = np.zeros((nch * 128, 128), NP_BF16)
        valid = lt >= 0
        ohb[np.nonzero(valid)[0], lt[valid]] = 1.0
        return (src_b.astype(np.int16).reshape(-1, 16).T,     # [16, nch*8]
                eb.reshape(nch, 128, C.ed).transpose(2, 0, 1),  # [ed, nch, 128]
                ohb.reshape(nch, 128, 128).transpose(1, 0, 2))  # [e, nch, t]

    cores = []
    for c in range(C.ncores):
        idxw = np.zeros((128, C.nblk, cpb * 8), np.int16)
        eeT = np.zeros((C.nblk, 128, cpb, 128), NP_BF16)
        oh = np.zeros((C.nblk, 128, cpb, 128), NP_BF16)
        for b in range(C.nblk):
            s1a, d1a, e1a, s2a, d2a, e2a = halves[c * C.nblk + b]
            i1, ee1, oh1 = pack_half(s1a, d1a, e1a, cpb1, 0)
            i2, ee2, oh2 = pack_half(s2a, d2a, e2a, cpb2, 1)
            base = np.concatenate([i1, i2], axis=1)   # [16, cpb*8]
            for k in range(8):
                idxw[k * 16:(k + 1) * 16, b, :] = base
            eeT[b, :, :cpb1], eeT[b, :, cpb1:] = ee1, ee2
            oh[b, :, :cpb1], oh[b, :, cpb1:] = oh1, oh2

        th = np.zeros((C.tgt_pad, D), np.float32)
        lo = c * C.tgt_per
        hi = min((c + 1) * C.tgt_per, C.n_tgt)
        th[:hi - lo] = np.asarray(tgt_h)[lo:hi] - 1.0   # elu trick: + (tgt_h - 1)
        sh = np.zeros((C.in_dim, C.tgt_pad), np.float32)
        lo2 = c * C.tgt_per
        hi2 = min((c + 1) * C.tgt_per, C.n_src)
        sh[:, :hi2 - lo2] = np.asarray(src_h)[lo2:hi2].T

        cores.append({
            "idxw": idxw, "eeT": eeT, "oh": oh,
            "tgt_hm1": th.astype(NP_BF16),
            "src_hT": sh.astype(NP_BF16),
        })

    shared = {
        "wfc": Wfc_p.astype(NP_BF16),
        "m1": M1.astype(NP_BF16),
        "v": V.astype(NP_BF16),
        "w1": np.asarray(w1).astype(NP_BF16),
        "w2": np.asarray(w2).astype(NP_BF16),
        "b1c": np.asarray(b1, np.float32).reshape(C.fh, 1),
        "b2_rep": np.tile(np.asarray(b2, np.float32).reshape(1, D), (128, 1)),
        "g_rep": np.tile(np.asarray(ln_g, np.float32).reshape(1, D), (128, 1)),
        "b_rep": np.tile(np.asarray(ln_b, np.float32).reshape(1, D), (128, 1)),
        "identb": np.eye(128, dtype=NP_BF16),
    }
    return cores, shared


def build_program(C):
    nc = bacc.Bacc("TRN2", target_bir_lowering=False, debug=False,
                   num_devices=C.ncores, num_swdge_queues=NQUEUES)
    H, O, D, NBLK = C.h, C.o, C.d, C.nblk
    CPB1, CPB2, CPB = C.cpb1, C.cpb2, C.cpb
    ZR = C.zrow
    NPAD = C.tgt_pad
    R2 = C.tgt_per - RSPLIT

    # -------- I/O --------
    def din(name, shape, dt):
        return nc.dram_tensor(name, shape, dt, kind="ExternalInput").ap()

    idxw = din("idxw", [128, NBLK, CPB * 8], I16)
    eeT = din("eeT", [NBLK, 128, CPB, 128], BF16)
    oh = din("oh", [NBLK, 128, CPB, 128], BF16)
    tgt_hm1 = din("tgt_hm1", [NPAD, D], BF16)
    src_hT = din("src_hT", [C.in_dim, NPAD], BF16)
    wfc = din("wfc", [C.in_dim, D], BF16)
    m1 = din("m1", [C.in_dim, H], BF16)
    vmat = din("v", [C.ed, H], BF16)
    w1 = din("w1", [D, C.fh], BF16)
    w2 = din("w2", [C.fh, D], BF16)
    b1c = din("b1c", [C.fh, 1], F32)
    b2_rep = din("b2_rep", [128, D], F32)
    g_rep = din("g_rep", [128, D], F32)
    b_rep = din("b_rep", [128, D], F32)
    identb = din("identb", [128, 128], BF16)

    out_shard = nc.dram_tensor("out_shard", [NPAD, D], F32,
                               kind="ExternalOutput").ap()

    zc_bounce = nc.dram_tensor("zc_bounce", [C.tgt_per, ZR], BF16).ap()
    zc_space = "Shared" if C.ncores > 4 else None
    zc1 = nc.dram_tensor("zc1", [C.ncores * RSPLIT, ZR], BF16,
                         addr_space=zc_space).ap()
    zc2 = nc.dram_tensor("zc2", [C.ncores * R2, ZR], BF16,
                         addr_space=zc_space).ap()
    dummy_out = nc.dram_tensor("dummy_out", [C.ncores, 32], BF16,
                               addr_space=zc_space).ap()
    dummy_src = nc.dram_tensor("dummy_src", [1, 32], BF16).ap()

    KT = C.in_dim // 128   # 4
    FT = D // 128          # 4
    MT1 = C.fh // 128      # 16
    chunks = [(s, min(512, NPAD - s)) for s in range(0, NPAD, 512)]

    with tile.TileContext(nc) as tc, ExitStack() as top:
        const = top.enter_context(tc.tile_pool(name="const", bufs=1))

        # skew-absorbing dummy collective
        dsrc_sb = const.tile([1, 32], BF16)
        nc.vector.memset(dsrc_sb[:], 0.0)
        nc.sync.dma_start(dummy_src[:, :], dsrc_sb[:])
        nc.gpsimd.collective_compute(
            "AllGather", mybir.AluOpType.bypass,
            replica_groups=[list(range(C.ncores))],
            ins=[dummy_src[:, :]], outs=[dummy_out[:, :]],
        )

        wfc_sb = const.tile([128, KT, D], BF16)
        nc.sync.dma_start(wfc_sb[:], wfc.rearrange("(kt p) m -> p kt m", p=128))
        m1_sb = const.tile([128, KT, H], BF16)
        nc.sync.dma_start(m1_sb[:], m1.rearrange("(kt p) m -> p kt m", p=128))
        v_sb = const.tile([128, H], BF16)
        nc.sync.dma_start(v_sb[:], vmat[:, :])
        idx_sb = const.tile([128, NBLK, CPB * 8], I16)
        nc.sync.dma_start(idx_sb[:], idxw[:, :, :])
        w1_sb = const.tile([128, KT, MT1, 128], BF16)
        w2_sb = const.tile([128, MT1, FT, 128], BF16)
        b1_sb = const.tile([128, MT1, 1], F32)
        b2r_sb = const.tile([128, D], F32)
        grep_sb = const.tile([128, D], F32)
        brep_sb = const.tile([128, D], F32)
        idb_sb = const.tile([128, 128], BF16)
        zero_sb = const.tile([128, 1], F32)
        nc.vector.memset(zero_sb[:], 0.0)
        eps_sb = const.tile([128, 1], F32)
        nc.vector.memset(eps_sb[:], LN_EPS)

        # ---------------- phase 0: z rows + s1 rows -> zc_bounce -> AllGather
        with ExitStack() as p0:
            ps0 = p0.enter_context(tc.tile_pool(name="ps0", bufs=2, space="PSUM"))
            zr_pool = p0.enter_context(tc.tile_pool(name="zrow", bufs=3))
            shp = p0.enter_context(tc.tile_pool(name="shp", bufs=1))
            sh_sb = shp.tile([128, KT, NPAD], BF16)
            nc.sync.dma_start(sh_sb[:],
                              src_hT.rearrange("(kt p) n -> p kt n", p=128))
            for nb in range(NBLK):
                rows = min(128, C.tgt_per - nb * 128)
                if rows <= 0:
                    break
                z_ps = ps0.tile([128, D], F32, tag="zps")
                for kt in range(KT):
                    nc.tensor.matmul(z_ps[:], sh_sb[:, kt, nb * 128:(nb + 1) * 128],
                                     wfc_sb[:, kt, :], start=(kt == 0),
                                     stop=(kt == KT - 1))
                s1_ps = ps0.tile([128, H], F32, tag="s1ps")
                for kt in range(KT):
                    nc.tensor.matmul(s1_ps[:], sh_sb[:, kt, nb * 128:(nb + 1) * 128],
                                     m1_sb[:, kt, :], start=(kt == 0),
                                     stop=(kt == KT - 1))
                zrow = zr_pool.tile([128, D + H], BF16, tag="zrow")
                nc.vector.tensor_copy(zrow[:, 0:D], z_ps[:])
                nc.vector.tensor_copy(zrow[:, D:D + H], s1_ps[:])
                nc.sync.dma_start(zc_bounce[nb * 128:nb * 128 + rows, 0:D + H],
                                  zrow[0:rows, :])

        nc.gpsimd.collective_compute(
            "AllGather", mybir.AluOpType.bypass,
            replica_groups=[list(range(C.ncores))],
            ins=[zc_bounce[0:RSPLIT, :]], outs=[zc1[:, :]],
        )
        nc.gpsimd.collective_compute(
            "AllGather", mybir.AluOpType.bypass,
            replica_groups=[list(range(C.ncores))],
            ins=[zc_bounce[RSPLIT:C.tgt_per, :]], outs=[zc2[:, :]],
        )

        # FFN/LN consts load after phase 0 so they don't delay the z pipeline
        nc.sync.dma_start(w1_sb[:], w1.rearrange("(kt p) (mt m) -> p kt mt m",
                                                 p=128, m=128))
        nc.sync.dma_start(w2_sb[:], w2.rearrange("(kt p) (mt m) -> p kt mt m",
                                                 p=128, m=128))
        nc.sync.dma_start(b1_sb[:], b1c.rearrange("(mt p) x -> p mt x", p=128))
        nc.sync.dma_start(b2r_sb[:], b2_rep[:, :])
        nc.sync.dma_start(grep_sb[:], g_rep[:, :])
        nc.sync.dma_start(brep_sb[:], b_rep[:, :])
        nc.sync.dma_start(idb_sb[:], identb[:, :])

        # ---------------- blocks + FFN, interleaved so PE fills gather gaps
        with ExitStack() as pb:
            ps_s2 = pb.enter_context(tc.tile_pool(name="ps_s2", bufs=1, space="PSUM"))
            ps_hag = pb.enter_context(tc.tile_pool(name="ps_hag", bufs=1, space="PSUM"))
            ps_tp = pb.enter_context(tc.tile_pool(name="ps_tp", bufs=1, space="PSUM"))
            ps_a1 = pb.enter_context(tc.tile_pool(name="ps_a1", bufs=2, space="PSUM"))
            ps_o2 = pb.enter_context(tc.tile_pool(name="ps_o2", bufs=2, space="PSUM"))
            gpool = pb.enter_context(tc.tile_pool(name="zg", bufs=2))
            zgsp = pb.enter_context(tc.tile_pool(name="zgs", bufs=2))
            epool = pb.enter_context(tc.tile_pool(name="escore", bufs=2))
            hpool = pb.enter_context(tc.tile_pool(name="hb", bufs=1))
            hkeep = pb.enter_context(tc.tile_pool(name="hb3", bufs=5))
            tgtp = pb.enter_context(tc.tile_pool(name="tgtp", bufs=2))
            hbtp = pb.enter_context(tc.tile_pool(name="hbt", bufs=1))
            r1p = pb.enter_context(tc.tile_pool(name="r1", bufs=1))
            tmpp = pb.enter_context(tc.tile_pool(name="tmp", bufs=2))
            lnp = pb.enter_context(tc.tile_pool(name="ln", bufs=1))
            stp = pb.enter_context(tc.tile_pool(name="stat", bufs=2))

            hbT = hbtp.tile([128, FT, NPAD], BF16)
            hb3_tiles = {}
            zg_tiles = {}

            qsel = [0]

            def emit_gathers(nb, half):
                if nb not in zg_tiles:
                    zg = gpool.tile([128, CPB, ZR], BF16, tag="zg")
                    zg_tiles[nb] = zg
                zg = zg_tiles[nb]
                lo, hi = (0, CPB1) if half == 0 else (CPB1, CPB)
                tab = zc1 if half == 0 else zc2
                for g0 in range(lo, hi, GMAX):
                    gn = min(GMAX, hi - g0)
                    nc.gpsimd.dma_gather(
                        out_ap=zg[:, g0:g0 + gn, :], in_ap=tab[:, :],
                        idxs_ap=idx_sb[:, nb, g0 * 8:(g0 + gn) * 8],
                        num_idxs=gn * 128, num_idxs_reg=gn * 128,
                        elem_size=ZR, queue_num=qsel[0] % NQUEUES)
                    qsel[0] += 1

            def emit_ffn_chunk(cs, cw):
                r1 = r1p.tile([128, MT1, cw], BF16, tag="r1")
                for mt in range(MT1):
                    a1 = ps_a1.tile([128, cw], F32, tag="a1")
                    for kt in range(KT):
                        nc.tensor.matmul(a1[:], w1_sb[:, kt, mt, :],
                                         hbT[:, kt, cs:cs + cw],
                                         start=(kt == 0), stop=(kt == KT - 1))
                    if mt % 2 == 0:
                        nc.scalar.activation(r1[:, mt, :], a1[:],
                                             mybir.ActivationFunctionType.Relu,
                                             bias=b1_sb[:, mt, :])
                    else:
                        nc.vector.tensor_scalar(r1[:, mt, :], a1[:],
                                                b1_sb[:, mt, :], 0.0,
                                                mybir.AluOpType.add,
                                                mybir.AluOpType.max)
                # o2 node-major: stationary = r1 node-chunk, moving = w2
                for ns in range(cw // 128):
                    nb_ln = (cs + ns * 128) // 128
                    o2 = ps_o2.tile([128, D], F32, tag="o2")
                    for kt2 in range(MT1):
                        nc.tensor.matmul(
                            o2[:], r1[:, kt2, ns * 128:(ns + 1) * 128],
                            w2_sb[:, kt2, :, :],
                            start=(kt2 == 0), stop=(kt2 == MT1 - 1))
                    # r2n = (o2 + b2) + hb3   (ffn out + bias + residual)
                    t1 = tmpp.tile([128, D], F32, tag="t1")
                    nc.vector.tensor_tensor(t1[:], o2[:], b2r_sb[:],
                                            mybir.AluOpType.add)
                    r2n = lnp.tile([128, D], F32, tag="r2n")
                    nc.vector.tensor_tensor(r2n[:], t1[:],
                                            hb3_tiles.pop(nb_ln)[:],
                                            mybir.AluOpType.add)
                    # LayerNorm via bn_stats
                    st6 = stp.tile([128, 6], F32, tag="st6")
                    nc.vector.bn_stats(st6[:], r2n[:])
                    mv = stp.tile([128, 2], F32, tag="mv")
                    nc.vector.bn_aggr(mv[:], st6[:])
                    std = stp.tile([128, 1], F32, tag="std")
                    nc.scalar.activation(std[:], mv[:, 1:2],
                                         mybir.ActivationFunctionType.Sqrt,
                                         bias=eps_sb[:, :])
                    rstd = stp.tile([128, 1], F32, tag="rstd")
                    nc.vector.reciprocal(rstd[:], std[:])
                    xn = lnp.tile([128, D], BF16, tag="xn")
                    nc.vector.tensor_scalar(xn[:], r2n[:], mv[:, 0:1], rstd[:],
                                            mybir.AluOpType.subtract,
                                            mybir.AluOpType.mult)
                    xg = lnp.tile([128, D], F32, tag="xg")
                    nc.vector.scalar_tensor_tensor(
                        xg[:], xn[:], 1.0, grep_sb[:],
                        mybir.AluOpType.mult, mybir.AluOpType.mult)
                    orow = lnp.tile([128, D], F32, tag="orow")
                    nc.vector.tensor_tensor(orow[:], xg[:], brep_sb[:],
                                            mybir.AluOpType.add)
                    nc.sync.dma_start(out_shard[nb_ln * 128:(nb_ln + 1) * 128, :],
                                      orow[:])

            def emit_block(nb):
                zg = zg_tiles[nb]
                ee_t = gpool.tile([128, CPB, 128], BF16, tag="ee")
                nc.sync.dma_start(ee_t[:], eeT[nb])
                oh_t = gpool.tile([128, CPB, 128], BF16, tag="oh")
                nc.sync.dma_start(oh_t[:], oh[nb])
                tgtb = tgtp.tile([128, D], BF16, tag="tgtb")
                nc.sync.dma_start(tgtb[:], tgt_hm1[nb * 128:(nb + 1) * 128, :])

                s2_ps = ps_s2.tile([128, CPB * H], F32, tag="s2")
                for j in range(CPB):
                    nc.tensor.matmul(s2_ps[:, j * H:(j + 1) * H], ee_t[:, j, :],
                                     v_sb[:, :], start=True, stop=True)
                e1 = epool.tile([128, CPB, H], F32, tag="e1")
                e2 = epool.tile([128, CPB, H], F32, tag="e2")
                eexp = epool.tile([128, CPB, H], BF16, tag="eexp")
                zgs = zgsp.tile([128, CPB, D], BF16, tag="zgs")
                for (lo, hi) in ((0, CPB1), (CPB1, CPB)):
                    w = hi - lo
                    nc.vector.tensor_tensor(
                        e1[:, lo:hi, :],
                        s2_ps[:, lo * H:hi * H].rearrange("p (c h) -> p c h", h=H),
                        zg[:, lo:hi, D:D + H],
                        mybir.AluOpType.add)
                    # leaky relu: max(x, 0.01*x) fused
                    nc.vector.scalar_tensor_tensor(
                        e2[:, lo:hi, :], e1[:, lo:hi, :], LEAK, e1[:, lo:hi, :],
                        mybir.AluOpType.mult, mybir.AluOpType.max)
                    nc.scalar.activation(eexp[:, lo:hi, :], e2[:, lo:hi, :],
                                         mybir.ActivationFunctionType.Exp,
                                         bias=zero_sb[:, :])
                    nc.vector.tensor_tensor(
                        zgs[:, lo:hi, :].rearrange("p c (o h) -> p c o h", h=H),
                        zg[:, lo:hi, 0:D].rearrange("p c (o h) -> p c o h", h=H),
                        eexp[:, lo:hi, :].rearrange("p c (h x) -> p c x h", x=1)
                            .broadcast_to([128, w, O, H]),
                        mybir.AluOpType.mult)

                hag = ps_hag.tile([128, D + H], F32, tag="hag")
                for j in range(CPB):
                    nc.tensor.matmul(hag[:, 0:D], oh_t[:, j, :], zgs[:, j, :],
                                     start=(j == 0), stop=(j == CPB - 1),
                                     skip_group_check=True)
                    nc.tensor.matmul(hag[:, D:D + H], oh_t[:, j, :], eexp[:, j, :],
                                     start=(j == 0), stop=(j == CPB - 1),
                                     skip_group_check=True)

                den = epool.tile([128, H], F32, tag="den")
                nc.vector.tensor_scalar_max(den[:], hag[:, D:D + H], 1e-30)
                rec = epool.tile([128, H], F32, tag="rec")
                nc.vector.reciprocal(rec[:], den[:])

                hbp = hpool.tile([128, D], BF16, tag="hbp")
                nc.vector.tensor_tensor(
                    hbp[:, :].rearrange("p (h o) -> p h o", o=O),
                    hag[:, 0:D].rearrange("p (o h) -> p h o", h=H),
                    rec[:, :].rearrange("p (h x) -> p h x", x=1)
                        .broadcast_to([128, H, O]),
                    mybir.AluOpType.mult)
                # elu(x) + tgt = max(x,0) + exp(min(x,0)) + (tgt-1)
                # exp(min(x,0)) = exp(-relu(-x)), both on the scalar engine
                nrx = hpool.tile([128, D], BF16, tag="nrx")
                nc.scalar.activation(nrx[:], hbp[:],
                                     mybir.ActivationFunctionType.Relu,
                                     bias=zero_sb[:, :], scale=-1.0)
                ex = hpool.tile([128, D], BF16, tag="ex")
                nc.scalar.activation(ex[:], nrx[:],
                                     mybir.ActivationFunctionType.Exp,
                                     bias=zero_sb[:, :], scale=-1.0)
                hb23 = hpool.tile([128, D], BF16, tag="hb23")
                nc.vector.scalar_tensor_tensor(
                    hb23[:], hbp[:], 0.0, tgtb[:],
                    mybir.AluOpType.max, mybir.AluOpType.add)
                hb3 = hkeep.tile([128, D], BF16, tag="hb3")
                hb3_tiles[nb] = hb3
                nc.vector.tensor_tensor(hb3[:], hb23[:], ex[:],
                                        mybir.AluOpType.add)
                for ft in range(FT):
                    tpb = ps_tp.tile([128, 128], BF16, tag="tp")
                    nc.tensor.transpose(tpb[:], hb3[:, ft * 128:(ft + 1) * 128],
                                        idb_sb[:])
                    nc.vector.tensor_copy(hbT[:, ft, nb * 128:(nb + 1) * 128],
                                          tpb[:])

            # pipeline: prefetch both blocks' half-1 gathers before block 0's
            # half-2 (which waits on the second AllGather)
            emit_gathers(0, 0)
            emit_gathers(1, 0)
            emit_gathers(0, 1)
            next_chunk = 0
            for nb in range(NBLK):
                if nb == 1:
                    emit_gathers(1, 1)
                elif nb >= 2:
                    emit_gathers(nb, 0)
                    emit_gathers(nb, 1)
                emit_block(nb)
                del zg_tiles[nb]

                while (next_chunk < len(chunks)
                       and chunks[next_chunk][0] + chunks[next_chunk][1]
                       <= (nb + 1) * 128):
                    cs, cw = chunks[next_chunk]
                    emit_ffn_chunk(cs, cw)
                    next_chunk += 1

    nc.compile()
    return nc


_CACHE = {}


def _get_program(C):
    key = (C.ncores, C.n_src, C.n_tgt, C.e, C.cpb1, C.cpb2)
    if key not in _CACHE:
        _CACHE[key] = build_program(C)
    return _CACHE[key]


def kernel(src_h, tgt_h, edge_embed, edge_src, edge_dst,
           W_fc, W_feat, attn_a, w1, b1, w2, b2, ln_g, ln_b):
    from concourse.bass_utils import run_bass_kernel_spmd

    C = full_cfg()
    cores, shared = host_prep(C, src_h, tgt_h, edge_embed, edge_src, edge_dst,
                              W_fc, W_feat, attn_a, w1, b1, w2, b2, ln_g, ln_b)
    nc = _get_program(C)
    in_maps = []
    for c in range(C.ncores):
        m = dict(shared)
        cc = cores[c]
        m.update(idxw=cc["idxw"], eeT=cc["eeT"], oh=cc["oh"],
                 tgt_hm1=cc["tgt_hm1"], src_hT=cc["src_hT"])
        in_maps.append(m)
    import os
    try:
        res = run_bass_kernel_spmd(nc, in_maps, list(range(C.ncores)))
    except Exception:
        if os.environ.get("BASS_TRACE"):
            os.environ["BASS_NEVER_TRACE"] = "1"
            res = run_bass_kernel_spmd(nc, in_maps, list(range(C.ncores)))
        else:
            raise
    global _last_results
    _last_results = res
    out = np.concatenate(
        [res.results[c]["out_shard"][:C.tgt_per] for c in range(C.ncores)], axis=0)
    return np.ascontiguousarray(out, dtype=np.float32)
